# revision 28
# baseline (speedup 1.0000x reference)
"""Trainium2 Bass kernel for nn_ArgumentGCN (GNN message passing).

B=16, N=512, D=256, K=12. Data-parallel over batch: 8 NeuronCores x 2 batches.
Per batch b:
    w      = sigmoid(node @ Ww.T + bw)                          [N]
    self_i = node @ Ws.T + bs                                   [N, D]
    Gt_k   = graphs_k * (1 - eye) * mask[col]                   [N, N]  (bf16, built on device)
    neigh  = clamp(mask[row] * rowsum(sum_k Gt_k), min 1)       [N]
    M_k    = Gt_k @ (w * mask * node)                           [N, D]
    agg    = sum_k M_k @ Wk[k].T                                [N, D]
    out    = relu(self_i + mask[row]/neigh * agg)

Device dataflow (per core):
  - graphs int32 DMA'd naturally, cast+masked+row-reduced by one DVE
    tensor_tensor_reduce per 128-row strip (neigh falls out of the reduction),
    transposed on-chip with a single DMA-xbar transpose per matrix,
    then two PSUM-accumulated matmul stages. Everything else is fused around it.
"""

import os
import sys

import numpy as np

for _p in ("/opt/trn_rl_repo",):
    if os.path.isdir(_p) and _p not in sys.path:
        sys.path.insert(0, _p)

import concourse.bass as bass
import concourse.tile as tile
from concourse import bacc, mybir
from concourse.bass_utils import run_bass_kernel_spmd

B, N, D, K = 16, 512, 256, 12
NCORES = 8
BLOC = B // NCORES          # batches per core
P = 128                     # partitions
MC = N // P                 # 4 row chunks of a graph matrix
DC = D // P                 # 2 chunks of the feature dim

F32 = mybir.dt.float32
BF16 = mybir.dt.bfloat16
I32 = mybir.dt.int32

ALU = mybir.AluOpType
ACTF = mybir.ActivationFunctionType

_CACHE = {}


def _body(tc, nc, d):
    """Emit the per-core kernel into TileContext tc."""
    import contextlib

    ctx = contextlib.ExitStack()
    const = ctx.enter_context(tc.tile_pool(name="const", bufs=1))
    pool = ctx.enter_context(tc.tile_pool(name="work", bufs=2))
    gbpool = ctx.enter_context(tc.tile_pool(name="gbf", bufs=5))
    gmpool = ctx.enter_context(tc.tile_pool(name="gmask", bufs=3))
    gtpool = ctx.enter_context(tc.tile_pool(name="gt", bufs=3))
    mtpool = ctx.enter_context(tc.tile_pool(name="mt", bufs=3))
    scr = ctx.enter_context(tc.tile_pool(name="scr", bufs=2))
    mtpsum = ctx.enter_context(tc.tile_pool(name="mtpsum", bufs=2, space="PSUM"))
    aggpsum = ctx.enter_context(tc.tile_pool(name="aggpsum", bufs=1, space="PSUM"))

    with ctx:
        # ---------------- constants / weights ----------------
        # Ws [e, d] -> WsT bf16 [d(part, 2 chunks), e(256)]; casting DMA f32->bf16
        ws_b = const.tile([P, DC, D], BF16)
        nc.gpsimd.dma_start(ws_b[:], d["Ws"].rearrange("(c p) d -> p c d", p=P))
        wsT = const.tile([P, DC, D], BF16)
        for ec in range(DC):
            nc.scalar.dma_start_transpose(wsT[:, :, ec * P:(ec + 1) * P], ws_b[:, ec, :])

        # Wk [k, e, d] -> wkT bf16 [d(part), k, dc, e(256)]; transposes are
        # emitted inside the first batch's k-loop so they don't serialize startup.
        wk_b = const.tile([P, K * DC, D], BF16)
        nc.gpsimd.dma_start(wk_b[:], d["Wk"].rearrange("k (c p) d -> p (k c) d", p=P))
        wkT = const.tile([P, K, DC, D], BF16)

        # Ww broadcast [128, 256] f32 ; bw broadcast [128,1] ; bs broadcast [128, 256]
        ww_row = const.tile([1, D], F32)
        nc.gpsimd.dma_start(ww_row[:], d["Ww"])
        ww_bc = const.tile([P, D], F32)
        nc.gpsimd.partition_broadcast(ww_bc[:], ww_row[:])
        bw_row = const.tile([1, 1], F32)
        nc.gpsimd.dma_start(bw_row[:], d["bw"].rearrange("(a b) -> a b", b=1))
        bw_bc = const.tile([P, 1], F32)
        nc.gpsimd.partition_broadcast(bw_bc[:], bw_row[:])
        bs_row = const.tile([1, D], F32)
        nc.gpsimd.dma_start(bs_row[:], d["bs"].rearrange("(a e) -> a e", a=1))
        bs_bc = const.tile([P, D], F32)
        nc.gpsimd.partition_broadcast(bs_bc[:], bs_row[:])

        # ---------------- per-batch ----------------
        for b in range(BLOC):
            # node natural [p, mc, d] f32 + bf16 (casting DMA for the bf16 copy)
            node_f = pool.tile([P, MC, D], F32, tag="node_f")
            nc.sync.dma_start(node_f[:], d["node"][b].rearrange("(c p) e -> p c e", p=P))
            node_b = pool.tile([P, MC, D], BF16, tag="node_b")
            nc.gpsimd.dma_start(node_b[:], d["node"][b].rearrange("(c p) e -> p c e", p=P))

            # nodeT bf16 [p, dc, m]
            nodeT = pool.tile([P, DC, N], BF16, tag="nodeT")
            for mc in range(MC):
                nc.scalar.dma_start_transpose(
                    nodeT[:, :, mc * P:(mc + 1) * P], node_b[:, mc, :]
                )

            # masks via casting DMA (SWDGE converts int32 -> float):
            # column layout [p, mc] f32 and row layout [1, N] bf16
            mask_f = pool.tile([P, MC], F32, tag="mask_f")
            nc.gpsimd.dma_start(mask_f[:], d["node_mask"][b].rearrange("(c p) -> p c", p=P))
            mrow_b = pool.tile([1, N], BF16, tag="mrow_b")
            nc.gpsimd.dma_start(mrow_b[:], d["node_mask"][b].rearrange("(a n) -> a n", a=1))

            # MASKMUL[p, mc, n] = mask[n] * (1 - eye)  (bf16)
            maskmul = pool.tile([P, MC, N], BF16, tag="maskmul")
            for mc in range(MC):
                nc.gpsimd.partition_broadcast(maskmul[:, mc, :], mrow_b[:])
                nc.gpsimd.affine_select(
                    out=maskmul[:, mc, :],
                    in_=maskmul[:, mc, :],
                    pattern=[[-1, N]],
                    compare_op=ALU.not_equal,
                    fill=0.0,
                    base=mc * P,
                    channel_multiplier=1,
                )

            # w = sigmoid(node @ Ww.T + bw)  — fp32 row-dot on DVE, sigmoid on ACT
            wpre = pool.tile([P, MC], F32, tag="wpre")
            for mc in range(MC):
                wdot_scr = scr.tile([P, D], F32, tag="wdot")
                nc.vector.scalar_tensor_tensor(
                    out=wdot_scr[:],
                    in0=node_f[:, mc, :],
                    scalar=1.0,
                    in1=ww_bc[:],
                    op0=ALU.mult,
                    op1=ALU.mult,
                    accum_out=wpre[:, mc:mc + 1],
                )
            w_col = pool.tile([P, MC], F32, tag="w_col")
            nc.scalar.activation(w_col[:], wpre[:], ACTF.Sigmoid, bias=bw_bc[:], scale=1.0)
            nc.gpsimd.dma_start(d["out_w"][b].rearrange("(c p) -> p c", p=P), w_col[:])

            # wnode bf16 [p, nc, d] = node * (w * mask)
            wm_b = pool.tile([P, MC], F32, tag="wm_b")
            nc.vector.tensor_mul(wm_b[:], w_col[:], mask_f[:])
            wnode = pool.tile([P, MC, D], BF16, tag="wnode")
            for mc in range(MC):
                nc.vector.tensor_scalar_mul(wnode[:, mc, :], node_b[:, mc, :], wm_b[:, mc:mc + 1])

            # self_info = node @ Ws.T + bs   (psum -> sbuf f32)
            ps_self = aggpsum.tile([P, MC, D], F32, tag="ps_self")
            for mc in range(MC):
                for dc in range(DC):
                    nc.tensor.matmul(
                        ps_self[:, mc, :],
                        nodeT[:, dc, mc * P:(mc + 1) * P],
                        wsT[:, dc, :],
                        start=(dc == 0),
                        stop=(dc == DC - 1),
                    )
            self_sb = pool.tile([P, MC, D], F32, tag="self_sb")
            for mc in range(MC):
                nc.vector.tensor_add(self_sb[:, mc, :], ps_self[:, mc, :], bs_bc[:])

            # ---------------- main loop over edge types ----------------
            neighparts = pool.tile([P, MC * K], F32, tag="neighparts")
            ps_aggs = [
                aggpsum.tile([P, D], F32, tag=f"ps_agg{mc}", name=f"ps_agg{mc}")
                for mc in range(MC)
            ]

            # ---- software-pipelined: front(k) = load/stt/transpose,
            # ----                     back(k)  = stage1/evac/stage2
            # tile_wait_until ticks force the per-engine instruction order so
            # iteration k's compute overlaps iteration k+1's cast/transpose.
            PIPE = 3
            gts = {}
            tick0 = b * (K + 4)

            def issue_load(k):
                with tc.tile_wait_until(max(0, tick0 + k - PIPE)):
                    g_raw = gbpool.tile([P, MC, N], BF16, tag="g_raw", name=f"g_raw{b}_{k}")
                    nc.gpsimd.dma_start(
                        g_raw[:], d["graphs"][k, b].rearrange("(c p) n -> p c n", p=P)
                    )
                return g_raw

            loads = {}
            for kk in range(min(PIPE, K)):
                loads[kk] = issue_load(kk)

            def emit_front(k):
                g_raw = loads.pop(k)
                if b == 0:
                    # just-in-time Wk[k] transpose prep, opposite ring parity to gT
                    with tc.tile_wait_until(max(0, tick0 + k - 1)):
                        wk_eng = nc.scalar if k % 2 == 0 else nc.sync
                        for ec in range(DC):
                            wk_eng.dma_start_transpose(
                                wkT[:, k, :, ec * P:(ec + 1) * P], wk_b[:, k * DC + ec, :]
                            )
                # mask + diag-zero + neighbor row-sum partial per strip
                with tc.tile_wait_until(max(0, tick0 + k - 1.5)):
                    g_b = gmpool.tile([P, MC, N], BF16, tag="g_b", name=f"g_b{b}_{k}")
                    for mc in range(MC):
                        nc.vector.scalar_tensor_tensor(
                            out=g_b[:, mc, :],
                            in0=g_raw[:, mc, :],
                            scalar=1.0,
                            in1=maskmul[:, mc, :],
                            op0=ALU.mult,
                            op1=ALU.mult,
                            accum_out=neighparts[:, mc * K + k:mc * K + k + 1],
                        )
                # xbar transpose: gT[p, mc, nc, f] = Gt[n=nc*128+p, m=mc*128+f]
                with tc.tile_wait_until(max(0, tick0 + k - 1.2)):
                    gT = gtpool.tile([P, MC, MC, P], BF16, tag="gT", name=f"gT{b}_{k}")
                    eng = nc.sync if k % 2 == 0 else nc.scalar
                    eng.dma_start_transpose(gT[:], g_b[:])
                gts[k] = gT

            def emit_back(k):
                gT = gts.pop(k)
                with tc.tile_wait_until(tick0 + k):
                    mt_b = mtpool.tile([P, DC, N], BF16, tag="mt_b", name=f"mt_b{b}_{k}")
                    for dc in range(DC):
                        ps_mt = mtpsum.tile([P, N], F32, tag="ps_mt", name=f"ps_mt{b}_{k}_{dc}")
                        for nch in range(MC):
                            nc.tensor.matmul(
                                ps_mt[:],
                                wnode[:, nch, dc * P:(dc + 1) * P],
                                gT[:, :, nch, :],
                                start=(nch == 0),
                                stop=(nch == MC - 1),
                            )
                        nc.vector.tensor_copy(mt_b[:, dc, :], ps_mt[:])
                    for mc in range(MC):
                        for dc in range(DC):
                            nc.tensor.matmul(
                                ps_aggs[mc][:],
                                mt_b[:, dc, mc * P:(mc + 1) * P],
                                wkT[:, k, dc, :],
                                start=(k == 0 and dc == 0),
                                stop=(k == K - 1 and dc == DC - 1),
                            )

            for k in range(K):
                if k + PIPE < K:
                    loads[k + PIPE] = issue_load(k + PIPE)
                emit_front(k)
                if k >= 1:
                    emit_back(k - 1)
            emit_back(K - 1)

            # ---------------- epilogue ----------------
            with tc.tile_wait_until(tick0 + K):
                neigh = pool.tile([P, MC], F32, tag="neigh")
                nc.vector.tensor_reduce(
                    neigh[:],
                    neighparts[:].rearrange("p (c k) -> p c k", k=K),
                    axis=mybir.AxisListType.X,
                    op=ALU.add,
                )
                scale_c = pool.tile([P, MC], F32, tag="scale_c")
                nc.vector.tensor_mul(scale_c[:], neigh[:], mask_f[:])
                nc.vector.tensor_scalar_max(scale_c[:], scale_c[:], 1.0)
                nc.vector.reciprocal(scale_c[:], scale_c[:])
                nc.vector.tensor_mul(scale_c[:], scale_c[:], mask_f[:])

                out_sb = pool.tile([P, MC, D], F32, tag="out_sb")
                for mc in range(MC):
                    nc.vector.scalar_tensor_tensor(
                        out=out_sb[:, mc, :],
                        in0=ps_aggs[mc][:],
                        scalar=scale_c[:, mc:mc + 1],
                        in1=self_sb[:, mc, :],
                        op0=ALU.mult,
                        op1=ALU.add,
                    )
                    nc.vector.tensor_scalar_max(out_sb[:, mc, :], out_sb[:, mc, :], 0.0)
                nc.scalar.dma_start(
                    d["out_node"][b].rearrange("(c p) e -> p c e", p=P), out_sb[:]
                )


def build():
    if "nc" in _CACHE:
        return _CACHE["nc"]
    nc = bacc.Bacc("TRN2", target_bir_lowering=False, debug=False)
    d = {
        "node": nc.dram_tensor("node", [BLOC, N, D], F32, kind="ExternalInput").ap(),
        "node_mask": nc.dram_tensor("node_mask", [BLOC, N], I32, kind="ExternalInput").ap(),
        "graphs": nc.dram_tensor("graphs", [K, BLOC, N, N], I32, kind="ExternalInput").ap(),
        "Ww": nc.dram_tensor("Ww", [1, D], F32, kind="ExternalInput").ap(),
        "bw": nc.dram_tensor("bw", [1], F32, kind="ExternalInput").ap(),
        "Ws": nc.dram_tensor("Ws", [D, D], F32, kind="ExternalInput").ap(),
        "bs": nc.dram_tensor("bs", [D], F32, kind="ExternalInput").ap(),
        "Wk": nc.dram_tensor("Wk", [K, D, D], F32, kind="ExternalInput").ap(),
        "out_node": nc.dram_tensor("out_node", [BLOC, N, D], F32, kind="ExternalOutput").ap(),
        "out_w": nc.dram_tensor("out_w", [BLOC, N], F32, kind="ExternalOutput").ap(),
    }
    with tile.TileContext(nc) as tc:
        _body(tc, nc, d)
    nc.compile()
    _CACHE["nc"] = nc
    return nc


def make_in_maps(inputs):
    node = np.ascontiguousarray(np.asarray(inputs["node"], dtype=np.float32))
    mask = np.ascontiguousarray(np.asarray(inputs["node_mask"], dtype=np.int32))
    graphs = np.asarray(inputs["graphs"], dtype=np.int32)
    Ww = np.ascontiguousarray(np.asarray(inputs["Ww"], dtype=np.float32))
    bw = np.ascontiguousarray(np.asarray(inputs["bw"], dtype=np.float32))
    Ws = np.ascontiguousarray(np.asarray(inputs["Ws"], dtype=np.float32))
    bs = np.ascontiguousarray(np.asarray(inputs["bs"], dtype=np.float32))
    Wk = np.ascontiguousarray(np.asarray(inputs["Wk"], dtype=np.float32))
    in_maps = []
    for i in range(NCORES):
        sl = slice(i * BLOC, (i + 1) * BLOC)
        in_maps.append({
            "node": np.ascontiguousarray(node[sl]),
            "node_mask": np.ascontiguousarray(mask[sl]),
            "graphs": np.ascontiguousarray(graphs[:, sl]),
            "Ww": Ww, "bw": bw, "Ws": Ws, "bs": bs, "Wk": Wk,
        })
    return in_maps


LAST_EXEC_TIME_NS = None
LAST_RESULTS = None


def _ensure_ntff_hook():
    """Install the axon NTFF profiling hook if the image's antenv lacks it."""
    try:
        from antenv.axon_hooks import get_axon_ntff_profile_hook  # noqa: F401
        return
    except ImportError:
        pass
    try:
        import types
        import antenv
        if "/root/.axon_site" not in sys.path and os.path.isdir("/root/.axon_site"):
            sys.path.insert(0, "/root/.axon_site")
        from trn_agent_boot.trn_boot import _ntff_profile_via_ctypes
        hook = _ntff_profile_via_ctypes("/opt/axon/libaxon_pjrt.so")
        mod = types.ModuleType("antenv.axon_hooks")
        state = {"hook": hook}
        mod.get_axon_ntff_profile_hook = lambda: state["hook"]
        mod.set_axon_ntff_profile_hook = lambda h: state.__setitem__("hook", h)
        sys.modules["antenv.axon_hooks"] = mod
        antenv.axon_hooks = mod
    except Exception as e:  # profiling is best-effort; execution still works
        print(f"NTFF hook install failed: {e}", file=sys.stderr)


def kernel(**inputs):
    global LAST_EXEC_TIME_NS, LAST_RESULTS
    nc = build()
    in_maps = make_in_maps(inputs)
    trace = bool(int(os.environ.get("KERNEL_TRACE", "0")))
    if trace:
        _ensure_ntff_hook()
    res = run_bass_kernel_spmd(nc, in_maps, core_ids=list(range(NCORES)), trace=trace)
    LAST_EXEC_TIME_NS = res.exec_time_ns
    LAST_RESULTS = res
    node_out = np.concatenate([r["out_node"] for r in res.results], axis=0)
    w_out = np.concatenate([r["out_w"] for r in res.results], axis=0)
    all_weight = w_out[:, None, :].astype(np.float32)
    return node_out.astype(np.float32), all_weight


# revision 31
# speedup vs baseline: 1.3468x; 1.3468x over previous
"""Trainium2 Bass kernel for nn_ArgumentGCN (GNN message passing).

B=16, N=512, D=256, K=12. Data-parallel over batch: 8 NeuronCores x 2 batches.
Per batch b:
    w      = sigmoid(node @ Ww.T + bw)                          [N]
    self_i = node @ Ws.T + bs                                   [N, D]
    Gt_k   = graphs_k * (1 - eye) * mask[col]                   [N, N]  (bf16, built on device)
    neigh  = clamp(mask[row] * rowsum(sum_k Gt_k), min 1)       [N]
    M_k    = Gt_k @ (w * mask * node)                           [N, D]
    agg    = sum_k M_k @ Wk[k].T                                [N, D]
    out    = relu(self_i + mask[row]/neigh * agg)

Device dataflow (per core):
  - graphs int32 DMA'd naturally, cast+masked+row-reduced by one DVE
    tensor_tensor_reduce per 128-row strip (neigh falls out of the reduction),
    transposed on-chip with a single DMA-xbar transpose per matrix,
    then two PSUM-accumulated matmul stages. Everything else is fused around it.
"""

import os
import sys

import numpy as np

for _p in ("/opt/trn_rl_repo",):
    if os.path.isdir(_p) and _p not in sys.path:
        sys.path.insert(0, _p)

import concourse.bass as bass
import concourse.tile as tile
from concourse import bacc, mybir
from concourse.bass_utils import run_bass_kernel_spmd

B, N, D, K = 16, 512, 256, 12
NCORES = 8
BLOC = B // NCORES          # batches per core
P = 128                     # partitions
MC = N // P                 # 4 row chunks of a graph matrix
DC = D // P                 # 2 chunks of the feature dim

F32 = mybir.dt.float32
BF16 = mybir.dt.bfloat16
I32 = mybir.dt.int32

ALU = mybir.AluOpType
ACTF = mybir.ActivationFunctionType

_CACHE = {}


def _body(tc, nc, d):
    """Emit the per-core kernel into TileContext tc."""
    import contextlib

    ctx = contextlib.ExitStack()
    const = ctx.enter_context(tc.tile_pool(name="const", bufs=1))
    pool = ctx.enter_context(tc.tile_pool(name="work", bufs=2))
    gbpool = ctx.enter_context(tc.tile_pool(name="gbf", bufs=5))
    gmpool = ctx.enter_context(tc.tile_pool(name="gmask", bufs=3))
    gtpool = ctx.enter_context(tc.tile_pool(name="gt", bufs=3))
    mtpool = ctx.enter_context(tc.tile_pool(name="mt", bufs=3))
    scr = ctx.enter_context(tc.tile_pool(name="scr", bufs=2))
    mtpsum = ctx.enter_context(tc.tile_pool(name="mtpsum", bufs=2, space="PSUM"))
    aggpsum = ctx.enter_context(tc.tile_pool(name="aggpsum", bufs=1, space="PSUM"))

    with ctx:
        # ---------------- constants / weights ----------------
        # Ws [e, d] -> WsT bf16 [d(part, 2 chunks), e(256)]; casting DMA f32->bf16
        ws_b = const.tile([P, DC, D], BF16)
        nc.gpsimd.dma_start(ws_b[:], d["Ws"].rearrange("(c p) d -> p c d", p=P))
        wsT = const.tile([P, DC, D], BF16)
        for ec in range(DC):
            nc.scalar.dma_start_transpose(wsT[:, :, ec * P:(ec + 1) * P], ws_b[:, ec, :])

        # Wk [k, e, d] -> wkT bf16 [d(part), k, dc, e(256)]; transposes are
        # emitted inside the first batch's k-loop so they don't serialize startup.
        wk_b = const.tile([P, K * DC, D], BF16)
        nc.gpsimd.dma_start(wk_b[:], d["Wk"].rearrange("k (c p) d -> p (k c) d", p=P))
        wkT = const.tile([P, K, DC, D], BF16)

        # Ww broadcast [128, 256] f32 ; bw broadcast [128,1] ; bs broadcast [128, 256]
        ww_row = const.tile([1, D], F32)
        nc.gpsimd.dma_start(ww_row[:], d["Ww"])
        ww_bc = const.tile([P, D], F32)
        nc.gpsimd.partition_broadcast(ww_bc[:], ww_row[:])
        bw_row = const.tile([1, 1], F32)
        nc.gpsimd.dma_start(bw_row[:], d["bw"].rearrange("(a b) -> a b", b=1))
        bw_bc = const.tile([P, 1], F32)
        nc.gpsimd.partition_broadcast(bw_bc[:], bw_row[:])
        bs_row = const.tile([1, D], F32)
        nc.gpsimd.dma_start(bs_row[:], d["bs"].rearrange("(a e) -> a e", a=1))
        bs_bc = const.tile([P, D], F32)
        nc.gpsimd.partition_broadcast(bs_bc[:], bs_row[:])

        # ---------------- per-batch ----------------
        for b in range(BLOC):
            # node natural [p, mc, d] f32 + bf16 (casting DMA for the bf16 copy)
            node_f = pool.tile([P, MC, D], F32, tag="node_f")
            nc.sync.dma_start(node_f[:], d["node"][b].rearrange("(c p) e -> p c e", p=P))
            node_b = pool.tile([P, MC, D], BF16, tag="node_b")
            nc.gpsimd.dma_start(node_b[:], d["node"][b].rearrange("(c p) e -> p c e", p=P))

            # nodeT bf16 [p, dc, m]
            nodeT = pool.tile([P, DC, N], BF16, tag="nodeT")
            for mc in range(MC):
                nc.scalar.dma_start_transpose(
                    nodeT[:, :, mc * P:(mc + 1) * P], node_b[:, mc, :]
                )

            # masks via casting DMA (SWDGE converts int32 -> float):
            # column layout [p, mc] f32 and row layout [1, N] bf16
            mask_f = pool.tile([P, MC], F32, tag="mask_f")
            nc.gpsimd.dma_start(mask_f[:], d["node_mask"][b].rearrange("(c p) -> p c", p=P))
            mrow_b = pool.tile([1, N], BF16, tag="mrow_b")
            nc.gpsimd.dma_start(mrow_b[:], d["node_mask"][b].rearrange("(a n) -> a n", a=1))

            # MASKMUL[p, mc, n] = mask[n] * (1 - eye)  (bf16)
            maskmul = pool.tile([P, MC, N], BF16, tag="maskmul")
            for mc in range(MC):
                nc.gpsimd.partition_broadcast(maskmul[:, mc, :], mrow_b[:])
                nc.gpsimd.affine_select(
                    out=maskmul[:, mc, :],
                    in_=maskmul[:, mc, :],
                    pattern=[[-1, N]],
                    compare_op=ALU.not_equal,
                    fill=0.0,
                    base=mc * P,
                    channel_multiplier=1,
                )

            # w = sigmoid(node @ Ww.T + bw)  — fp32 row-dot on DVE, sigmoid on ACT
            wpre = pool.tile([P, MC], F32, tag="wpre")
            for mc in range(MC):
                wdot_scr = scr.tile([P, D], F32, tag="wdot")
                nc.vector.scalar_tensor_tensor(
                    out=wdot_scr[:],
                    in0=node_f[:, mc, :],
                    scalar=1.0,
                    in1=ww_bc[:],
                    op0=ALU.mult,
                    op1=ALU.mult,
                    accum_out=wpre[:, mc:mc + 1],
                )
            w_col = pool.tile([P, MC], F32, tag="w_col")
            nc.scalar.activation(w_col[:], wpre[:], ACTF.Sigmoid, bias=bw_bc[:], scale=1.0)
            nc.gpsimd.dma_start(d["out_w"][b].rearrange("(c p) -> p c", p=P), w_col[:])

            # wnode bf16 [p, nc, d] = node * (w * mask)
            wm_b = pool.tile([P, MC], F32, tag="wm_b")
            nc.vector.tensor_mul(wm_b[:], w_col[:], mask_f[:])
            wnode = pool.tile([P, MC, D], BF16, tag="wnode")
            for mc in range(MC):
                nc.vector.tensor_scalar_mul(wnode[:, mc, :], node_b[:, mc, :], wm_b[:, mc:mc + 1])

            # self_info = node @ Ws.T + bs   (psum -> sbuf f32)
            ps_self = aggpsum.tile([P, MC, D], F32, tag="ps_self")
            for mc in range(MC):
                for dc in range(DC):
                    nc.tensor.matmul(
                        ps_self[:, mc, :],
                        nodeT[:, dc, mc * P:(mc + 1) * P],
                        wsT[:, dc, :],
                        start=(dc == 0),
                        stop=(dc == DC - 1),
                    )
            self_sb = pool.tile([P, MC, D], F32, tag="self_sb")
            for mc in range(MC):
                nc.vector.tensor_add(self_sb[:, mc, :], ps_self[:, mc, :], bs_bc[:])

            # ---------------- main loop over edge types ----------------
            neighparts = pool.tile([P, MC * K], F32, tag="neighparts")
            ps_aggs = [
                aggpsum.tile([P, D], F32, tag=f"ps_agg{mc}", name=f"ps_agg{mc}")
                for mc in range(MC)
            ]

            # ---- software-pipelined over k ----
            # The transpose consumes the RAW casting-DMA output, so it has no
            # DVE dependency; the diagonal is zeroed post-transpose on GpSimd.
            # The neigh mask-pass (stt) feeds only the epilogue, off the
            # critical path.
            PIPE = 3
            gts = {}

            def issue_load(k):
                g_raw = gbpool.tile([P, MC, N], BF16, tag="g_raw", name=f"g_raw{b}_{k}")
                nc.gpsimd.dma_start(
                    g_raw[:], d["graphs"][k, b].rearrange("(c p) n -> p c n", p=P)
                )
                return g_raw

            loads = {}
            for kk in range(min(PIPE, K)):
                loads[kk] = issue_load(kk)

            def emit_front(k):
                g_raw = loads[k]
                if b == 0:
                    # just-in-time Wk[k] transpose prep, opposite ring parity to gT
                    wk_eng = nc.scalar if k % 2 == 0 else nc.sync
                    for ec in range(DC):
                        wk_eng.dma_start_transpose(
                            wkT[:, k, :, ec * P:(ec + 1) * P], wk_b[:, k * DC + ec, :]
                        )
                # zero the diagonal in place on the natural layout (GpSimd),
                # then xbar-transpose: gT[p, mc, nc, f] = graphs[m=mc*128+f, n=nc*128+p]
                for c in range(MC):
                    nc.gpsimd.affine_select(
                        out=g_raw[:, c, :],
                        in_=g_raw[:, c, :],
                        pattern=[[-1, N]],
                        compare_op=ALU.not_equal,
                        fill=0.0,
                        base=c * P,
                        channel_multiplier=1,
                    )
                gT = gtpool.tile([P, MC, MC, P], BF16, tag="gT", name=f"gT{b}_{k}")
                eng = nc.sync if k % 2 == 0 else nc.scalar
                eng.dma_start_transpose(gT[:], g_raw[:])
                gts[k] = gT

            def emit_neigh(k):
                # neighbor-count partials; output product is discarded
                g_raw = loads.pop(k)
                g_scr = gmpool.tile([P, MC, N], BF16, tag="g_scr", name=f"g_scr{b}_{k}")
                for mc in range(MC):
                    nc.vector.scalar_tensor_tensor(
                        out=g_scr[:, mc, :],
                        in0=g_raw[:, mc, :],
                        scalar=1.0,
                        in1=maskmul[:, mc, :],
                        op0=ALU.mult,
                        op1=ALU.mult,
                        accum_out=neighparts[:, mc * K + k:mc * K + k + 1],
                    )

            def emit_back(k):
                gT = gts.pop(k)
                mt_b = mtpool.tile([P, DC, N], BF16, tag="mt_b", name=f"mt_b{b}_{k}")
                for dc in range(DC):
                    ps_mt = mtpsum.tile([P, N], F32, tag="ps_mt", name=f"ps_mt{b}_{k}_{dc}")
                    for nch in range(MC):
                        nc.tensor.matmul(
                            ps_mt[:],
                            wnode[:, nch, dc * P:(dc + 1) * P],
                            gT[:, :, nch, :],
                            start=(nch == 0),
                            stop=(nch == MC - 1),
                        )
                    nc.vector.tensor_copy(mt_b[:, dc, :], ps_mt[:])
                for mc in range(MC):
                    for dc in range(DC):
                        nc.tensor.matmul(
                            ps_aggs[mc][:],
                            mt_b[:, dc, mc * P:(mc + 1) * P],
                            wkT[:, k, dc, :],
                            start=(k == 0 and dc == 0),
                            stop=(k == K - 1 and dc == DC - 1),
                        )

            for k in range(K):
                if k + PIPE < K:
                    loads[k + PIPE] = issue_load(k + PIPE)
                emit_front(k)
                emit_neigh(k)
                if k >= 1:
                    emit_back(k - 1)
            emit_back(K - 1)

            # ---------------- epilogue ----------------
            if True:
                neigh = pool.tile([P, MC], F32, tag="neigh")
                nc.vector.tensor_reduce(
                    neigh[:],
                    neighparts[:].rearrange("p (c k) -> p c k", k=K),
                    axis=mybir.AxisListType.X,
                    op=ALU.add,
                )
                scale_c = pool.tile([P, MC], F32, tag="scale_c")
                nc.vector.tensor_mul(scale_c[:], neigh[:], mask_f[:])
                nc.vector.tensor_scalar_max(scale_c[:], scale_c[:], 1.0)
                nc.vector.reciprocal(scale_c[:], scale_c[:])
                nc.vector.tensor_mul(scale_c[:], scale_c[:], mask_f[:])

                out_sb = pool.tile([P, MC, D], F32, tag="out_sb")
                for mc in range(MC):
                    nc.vector.scalar_tensor_tensor(
                        out=out_sb[:, mc, :],
                        in0=ps_aggs[mc][:],
                        scalar=scale_c[:, mc:mc + 1],
                        in1=self_sb[:, mc, :],
                        op0=ALU.mult,
                        op1=ALU.add,
                    )
                    nc.vector.tensor_scalar_max(out_sb[:, mc, :], out_sb[:, mc, :], 0.0)
                nc.scalar.dma_start(
                    d["out_node"][b].rearrange("(c p) e -> p c e", p=P), out_sb[:]
                )


def build():
    if "nc" in _CACHE:
        return _CACHE["nc"]
    nc = bacc.Bacc("TRN2", target_bir_lowering=False, debug=False)
    d = {
        "node": nc.dram_tensor("node", [BLOC, N, D], F32, kind="ExternalInput").ap(),
        "node_mask": nc.dram_tensor("node_mask", [BLOC, N], I32, kind="ExternalInput").ap(),
        "graphs": nc.dram_tensor("graphs", [K, BLOC, N, N], I32, kind="ExternalInput").ap(),
        "Ww": nc.dram_tensor("Ww", [1, D], F32, kind="ExternalInput").ap(),
        "bw": nc.dram_tensor("bw", [1], F32, kind="ExternalInput").ap(),
        "Ws": nc.dram_tensor("Ws", [D, D], F32, kind="ExternalInput").ap(),
        "bs": nc.dram_tensor("bs", [D], F32, kind="ExternalInput").ap(),
        "Wk": nc.dram_tensor("Wk", [K, D, D], F32, kind="ExternalInput").ap(),
        "out_node": nc.dram_tensor("out_node", [BLOC, N, D], F32, kind="ExternalOutput").ap(),
        "out_w": nc.dram_tensor("out_w", [BLOC, N], F32, kind="ExternalOutput").ap(),
    }
    with tile.TileContext(nc) as tc:
        _body(tc, nc, d)
    nc.compile()
    _CACHE["nc"] = nc
    return nc


def make_in_maps(inputs):
    node = np.ascontiguousarray(np.asarray(inputs["node"], dtype=np.float32))
    mask = np.ascontiguousarray(np.asarray(inputs["node_mask"], dtype=np.int32))
    graphs = np.asarray(inputs["graphs"], dtype=np.int32)
    Ww = np.ascontiguousarray(np.asarray(inputs["Ww"], dtype=np.float32))
    bw = np.ascontiguousarray(np.asarray(inputs["bw"], dtype=np.float32))
    Ws = np.ascontiguousarray(np.asarray(inputs["Ws"], dtype=np.float32))
    bs = np.ascontiguousarray(np.asarray(inputs["bs"], dtype=np.float32))
    Wk = np.ascontiguousarray(np.asarray(inputs["Wk"], dtype=np.float32))
    in_maps = []
    for i in range(NCORES):
        sl = slice(i * BLOC, (i + 1) * BLOC)
        in_maps.append({
            "node": np.ascontiguousarray(node[sl]),
            "node_mask": np.ascontiguousarray(mask[sl]),
            "graphs": np.ascontiguousarray(graphs[:, sl]),
            "Ww": Ww, "bw": bw, "Ws": Ws, "bs": bs, "Wk": Wk,
        })
    return in_maps


LAST_EXEC_TIME_NS = None
LAST_RESULTS = None


def _ensure_ntff_hook():
    """Install the axon NTFF profiling hook if the image's antenv lacks it."""
    try:
        from antenv.axon_hooks import get_axon_ntff_profile_hook  # noqa: F401
        return
    except ImportError:
        pass
    try:
        import types
        import antenv
        if "/root/.axon_site" not in sys.path and os.path.isdir("/root/.axon_site"):
            sys.path.insert(0, "/root/.axon_site")
        from trn_agent_boot.trn_boot import _ntff_profile_via_ctypes
        hook = _ntff_profile_via_ctypes("/opt/axon/libaxon_pjrt.so")
        mod = types.ModuleType("antenv.axon_hooks")
        state = {"hook": hook}
        mod.get_axon_ntff_profile_hook = lambda: state["hook"]
        mod.set_axon_ntff_profile_hook = lambda h: state.__setitem__("hook", h)
        sys.modules["antenv.axon_hooks"] = mod
        antenv.axon_hooks = mod
    except Exception as e:  # profiling is best-effort; execution still works
        print(f"NTFF hook install failed: {e}", file=sys.stderr)


def kernel(**inputs):
    global LAST_EXEC_TIME_NS, LAST_RESULTS
    nc = build()
    in_maps = make_in_maps(inputs)
    trace = bool(int(os.environ.get("KERNEL_TRACE", "0")))
    if trace:
        _ensure_ntff_hook()
    res = run_bass_kernel_spmd(nc, in_maps, core_ids=list(range(NCORES)), trace=trace)
    LAST_EXEC_TIME_NS = res.exec_time_ns
    LAST_RESULTS = res
    node_out = np.concatenate([r["out_node"] for r in res.results], axis=0)
    w_out = np.concatenate([r["out_w"] for r in res.results], axis=0)
    all_weight = w_out[:, None, :].astype(np.float32)
    return node_out.astype(np.float32), all_weight
